# revision 24
# baseline (speedup 1.0000x reference)
"""Bass/Trainium2 kernel for nn_BilinearDecoder.

Computes, for each edge e:
    out[e] = sigmoid( z[src[e]] . (z[dst[e]] @ W) )
with z: [N, 128] f32, edge_index: [2, E] int64, W: [128, 128] f32.

Strategy (8 NeuronCores, SPMD):
  - The hot loop is a random gather of two 512B z-rows per edge via the
    SWDGE dma_gather instruction (int16 indices). Since N=100000 > 32767,
    z is split into 4 row-slabs of 25000 and edges are bucketed host-side
    by their (src_slab, dst_slab) pair; within a bucket every index is
    slab-relative and fits in int16.
  - Each of the 16 global buckets is padded to a multiple of 8*128 with fake
    (valid) edges and split evenly across the 8 cores, so a single SPMD
    program (traced per-invocation with the actual bucket sizes) serves all
    cores.
  - Descriptor generation on the GpSimd (Q7) engine is the bottleneck
    (~8 ns/descriptor per SWDGE queue): each dma_gather runs on Q7 core
    pair (2q, 2q+1) of its queue q, so the program uses num_swdge_queues=4
    and deals gathers round-robin over the 4 queues so 4 core pairs emit
    descriptors concurrently.  Buckets are further split into chunks of
    <= 13 etile columns (1664 indices per gather; the final bucket tapers
    to 7) — the fine grain keeps all 4 pairs staggered and fed, and an
    8-deep gather tile pool removes all steady-state sequencer waits.
    num_idxs registers are hoisted (one MOVE per distinct gather size).
  - Per chunk and block of up to 4 etiles (128 edges each): PE transpose of
    the dst rows, matmul with W, then DVE multiply + segmented reduce for
    the per-edge dot product.  Sigmoid is batched on the scalar engine at
    the end; one contiguous DMA writes the [128, total_cols] logits out.
  - Edge j of a core's bucket lives at grid position (j % 128, j // 128);
    the host applies the inverse bucket permutation on the returned grid.

Measured on 8 trn2 cores: 377.6 us (baseline at session start: 1304 us).
"""

import numpy as np

N_NODES = 100000
LATENT = 128
N_EDGES = 625000
N_CORES = 8
SLAB = 25000          # z rows per slab; slab-relative idx fits int16
N_SLABS = 4
BLK = 4               # etile columns per compute block


def _build_nc(n_nodes, slab, bucket_cols, blk=BLK):
    """Trace the SPMD Bass program.

    bucket_cols: list of (slab_src, slab_dst, cols) with cols = per-core
    etile columns for that bucket (cols*128 edges per core).
    """
    import concourse.bacc as bacc
    import concourse.mybir as mybir
    import concourse.tile as tile

    f32 = mybir.dt.float32
    i16 = mybir.dt.int16

    total_cols = sum(c for _, _, c in bucket_cols)
    icols_total = 8 * total_cols  # int16 idx cols (16-wrap) per side

    nc = bacc.Bacc(
        "TRN2", target_bir_lowering=False, debug=False, num_swdge_queues=4
    )

    z = nc.dram_tensor("z", [n_nodes, LATENT], f32, kind="ExternalInput")
    w = nc.dram_tensor("w", [LATENT, LATENT], f32, kind="ExternalInput")
    ident = nc.dram_tensor("ident", [128, 128], f32, kind="ExternalInput")
    src16 = nc.dram_tensor("src16", [128, icols_total], i16, kind="ExternalInput")
    dst16 = nc.dram_tensor("dst16", [128, icols_total], i16, kind="ExternalInput")
    out = nc.dram_tensor("out", [128, total_cols], f32, kind="ExternalOutput")

    with tile.TileContext(nc) as tc:
        with (
            tc.tile_pool(name="const", bufs=1) as constp,
            tc.tile_pool(name="gather", bufs=8) as gatherp,
            tc.tile_pool(name="work", bufs=3) as workp,
            tc.tile_pool(name="psum", bufs=2, space="PSUM") as psump,
            tc.tile_pool(name="outp", bufs=1) as outp,
        ):
            # Index tables first (the first gathers wait on them); each in
            # two halves so early gathers unblock before the tail loads.
            srci = constp.tile([128, icols_total], i16)
            dsti = constp.tile([128, icols_total], i16)
            i0 = min(416, icols_total)   # ~4 chunks of indices
            ih = max(i0, (icols_total // 2) // 8 * 8)
            nc.sync.dma_start(srci[:, :i0], src16[:, :i0])
            nc.scalar.dma_start(dsti[:, :i0], dst16[:, :i0])
            nc.sync.dma_start(srci[:, i0:ih], src16[:, i0:ih])
            nc.scalar.dma_start(dsti[:, i0:ih], dst16[:, i0:ih])
            nc.sync.dma_start(srci[:, ih:], src16[:, ih:])
            nc.scalar.dma_start(dsti[:, ih:], dst16[:, ih:])
            w_sb = constp.tile([128, 128], f32)
            nc.sync.dma_start(w_sb[:], w[:])
            id_sb = constp.tile([128, 128], f32)
            nc.sync.dma_start(id_sb[:], ident[:])

            logits = outp.tile([128, total_cols], f32)

            # Split each bucket into chunks of at most CHUNK_COLS etile
            # columns; every gather goes to the next SWDGE queue round-robin.
            # Each queue q runs its descriptor generation on Q7 core pair
            # (2q, 2q+1), so 4 gathers can generate descriptors
            # concurrently, and the small grain keeps all pairs fed.
            CHUNK_COLS = 13
            nonempty = [bc for bc in bucket_cols if bc[2] > 0]
            chunks = []   # (slab_src, slab_dst, col0, cols)
            col0 = 0
            for bi, (a, d, cols) in enumerate(nonempty):
                # Taper both ends: tiny first chunks fill the 4-queue
                # pipeline fast (less ring-drain waiting at startup), and
                # small final chunks keep the trailing compute drain short.
                off = 0
                while off < cols:
                    if bi == 0 and off < 24:
                        step = 4
                    elif bi == len(nonempty) - 1:
                        step = 7
                    else:
                        step = CHUNK_COLS
                    cw = min(step, cols - off)
                    chunks.append((a, d, col0 + off, cw))
                    off += cw
                col0 += cols

            nidx_regs = {
                k: nc.gpsimd.to_reg(k)
                for k in sorted({cw * 128 for _, _, _, cw in chunks})
            }

            qn = 0
            for a, d, ccol0, cols in chunks:
                n_idx = cols * 128
                ic0, ic1 = 8 * ccol0, 8 * (ccol0 + cols)

                # idx j -> tile[j%128, (j//128)*128 : ...+128]
                zi = gatherp.tile([128, cols * 128], f32, tag="zi")
                nc.gpsimd.dma_gather(
                    out_ap=zi[:].rearrange("p (c f) -> p c f", f=128),
                    in_ap=z[a * slab:(a + 1) * slab, :],
                    idxs_ap=srci[:, ic0:ic1],
                    num_idxs=n_idx,
                    num_idxs_reg=nidx_regs[n_idx],
                    elem_size=128,
                    single_packet=False,
                    queue_num=qn % 4,
                )
                qn += 1
                zj = gatherp.tile([128, cols * 128], f32, tag="zj")
                nc.gpsimd.dma_gather(
                    out_ap=zj[:].rearrange("p (c f) -> p c f", f=128),
                    in_ap=z[d * slab:(d + 1) * slab, :],
                    idxs_ap=dsti[:, ic0:ic1],
                    num_idxs=n_idx,
                    num_idxs_reg=nidx_regs[n_idx],
                    elem_size=128,
                    single_packet=False,
                    queue_num=qn % 4,
                )
                qn += 1

                for b0 in range(0, cols, blk):
                    bw = min(blk, cols - b0)
                    # Transpose bw etiles of zj into one PSUM bank.
                    zjT_ps = psump.tile([128, blk * 128], f32, tag="zjT")
                    for c in range(bw):
                        nc.tensor.transpose(
                            zjT_ps[:, c * 128:(c + 1) * 128],
                            zj[:, (b0 + c) * 128:(b0 + c + 1) * 128],
                            id_sb[:],
                        )
                    zjT_sb = workp.tile([128, blk * 128], f32, tag="zjT_sb")
                    nc.scalar.copy(
                        zjT_sb[:, :bw * 128], zjT_ps[:, :bw * 128]
                    )

                    # u = zj @ W per etile: [e, f] in PSUM.
                    u_ps = psump.tile([128, blk * 128], f32, tag="u")
                    for c in range(bw):
                        nc.tensor.matmul(
                            u_ps[:, c * 128:(c + 1) * 128],
                            lhsT=zjT_sb[:, c * 128:(c + 1) * 128],
                            rhs=w_sb[:],
                            start=True,
                            stop=True,
                        )

                    # Per-edge dot: logits[:, t] = sum_f(u * zi).
                    vscr = workp.tile([128, blk * 128], f32, tag="v")
                    nc.vector.tensor_tensor(
                        out=vscr[:, :bw * 128],
                        in0=u_ps[:, :bw * 128],
                        in1=zi[:, b0 * 128:(b0 + bw) * 128],
                        op=mybir.AluOpType.mult,
                    )
                    nc.vector.tensor_reduce(
                        out=logits[:, ccol0 + b0:ccol0 + b0 + bw],
                        in_=vscr[:, :bw * 128].rearrange(
                            "p (c f) -> p c f", f=128
                        ),
                        axis=mybir.AxisListType.X,
                        op=mybir.AluOpType.add,
                    )

            sig = outp.tile([128, total_cols], f32)
            hh = total_cols // 2
            nc.scalar.activation(
                sig[:, :hh], logits[:, :hh],
                mybir.ActivationFunctionType.Sigmoid,
            )
            nc.sync.dma_start(out[:, :hh], sig[:, :hh])
            nc.scalar.activation(
                sig[:, hh:], logits[:, hh:],
                mybir.ActivationFunctionType.Sigmoid,
            )
            nc.sync.dma_start(out[:, hh:], sig[:, hh:])

    nc.compile()
    return nc


def _wrap16(idx_1d):
    """[n] int16 -> [128, n//16] int16: j at [j%16, j//16], replicated x8."""
    n = idx_1d.shape[0]
    assert n % 16 == 0
    w = idx_1d.reshape(n // 16, 16).T  # [16, n//16]
    return np.ascontiguousarray(np.tile(w, (8, 1)))


def _host_prep(z, edge_index, W, n_nodes=N_NODES, slab=SLAB, n_cores=N_CORES):
    """Bucket + shard the edges. Returns (bucket_cols, in_maps, gather_info)
    where gather_info lets the caller scatter per-core outputs back."""
    z = np.ascontiguousarray(np.asarray(z, dtype=np.float32))
    W = np.ascontiguousarray(np.asarray(W, dtype=np.float32))
    ei = np.asarray(edge_index)
    src = np.asarray(ei[0], dtype=np.int64)
    dst = np.asarray(ei[1], dtype=np.int64)
    n_edges = src.shape[0]
    ident = np.eye(128, dtype=np.float32)

    bucket = (src // slab) * N_SLABS + (dst // slab)
    perm = np.argsort(bucket, kind="stable")
    counts = np.bincount(bucket, minlength=N_SLABS * N_SLABS)

    grain = n_cores * 128
    bucket_cols = []          # (slab_src, slab_dst, per-core cols)
    src_parts, dst_parts = [], []   # per-bucket padded slab-relative indices
    edge_ids = []             # per-bucket padded original edge ids (-1 = pad)
    off = 0
    for b in range(N_SLABS * N_SLABS):
        a, d = divmod(b, N_SLABS)
        n_b = int(counts[b])
        g_b = ((n_b + grain - 1) // grain) * grain
        if n_b == 0:
            bucket_cols.append((a, d, 0))
            continue
        ids = perm[off:off + n_b]
        off += n_b
        s_rel = np.zeros(g_b, dtype=np.int16)
        d_rel = np.zeros(g_b, dtype=np.int16)
        e_id = np.full(g_b, -1, dtype=np.int64)
        s_rel[:n_b] = (src[ids] - a * slab).astype(np.int16)
        d_rel[:n_b] = (dst[ids] - d * slab).astype(np.int16)
        e_id[:n_b] = ids
        src_parts.append(s_rel)
        dst_parts.append(d_rel)
        edge_ids.append(e_id)
        bucket_cols.append((a, d, g_b // grain))

    in_maps = []
    core_edge_ids = []  # per core: concat of bucket slices' edge ids
    for k in range(n_cores):
        s_list, d_list, id_list = [], [], []
        pi = 0
        for (a, d, cols) in bucket_cols:
            if cols == 0:
                continue
            per_core = cols * 128
            sl = slice(k * per_core, (k + 1) * per_core)
            s_list.append(_wrap16(src_parts[pi][sl]))
            d_list.append(_wrap16(dst_parts[pi][sl]))
            id_list.append(edge_ids[pi][sl])
            pi += 1
        in_maps.append({
            "z": z,
            "w": W,
            "ident": ident,
            "src16": np.concatenate(s_list, axis=1),
            "dst16": np.concatenate(d_list, axis=1),
        })
        core_edge_ids.append(np.concatenate(id_list))

    return bucket_cols, in_maps, core_edge_ids


def _unshard(results, core_edge_ids, n_edges):
    """Scatter per-core [128, total_cols] grids back to the full edge order."""
    full = np.zeros(n_edges, dtype=np.float32)
    for k, res in enumerate(results):
        grid = np.asarray(res["out"])            # [128, total_cols]
        flat = grid.T.reshape(-1)                # edge j = t*128 + p
        ids = core_edge_ids[k]
        valid = ids >= 0
        full[ids[valid]] = flat[valid]
    return full


def kernel(z, edge_index, W, _trace=False):
    from concourse.bass_utils import run_bass_kernel_spmd

    bucket_cols, in_maps, core_edge_ids = _host_prep(z, edge_index, W)
    nc = _build_nc(N_NODES, SLAB, bucket_cols)
    res = run_bass_kernel_spmd(
        nc, in_maps, core_ids=list(range(N_CORES)), trace=_trace
    )
    n_edges = np.asarray(edge_index).shape[1]
    full = _unshard(res.results, core_edge_ids, n_edges)
    if _trace:
        kernel.last_results = res
    return full



# revision 25
# speedup vs baseline: 1.0321x; 1.0321x over previous
"""Bass/Trainium2 kernel for nn_BilinearDecoder.

Computes, for each edge e:
    out[e] = sigmoid( z[src[e]] . (z[dst[e]] @ W) )
with z: [N, 128] f32, edge_index: [2, E] int64, W: [128, 128] f32.

Strategy (8 NeuronCores, SPMD):
  - The hot loop is a random gather of two 512B z-rows per edge via the
    SWDGE dma_gather instruction (int16 indices). Since N=100000 > 32767,
    z is split into 4 row-slabs of 25000 and edges are bucketed host-side
    by their (src_slab, dst_slab) pair; within a bucket every index is
    slab-relative and fits in int16.
  - Each of the 16 global buckets is padded to a multiple of 8*128 with fake
    (valid) edges and split evenly across the 8 cores, so a single SPMD
    program (traced per-invocation with the actual bucket sizes) serves all
    cores.
  - Descriptor generation on the GpSimd (Q7) engine is the bottleneck
    (~8 ns/descriptor per SWDGE queue): each dma_gather runs on Q7 core
    pair (2q, 2q+1) of its queue q, so the program uses num_swdge_queues=4
    and deals gathers round-robin over the 4 queues so 4 core pairs emit
    descriptors concurrently.  Buckets are further split into chunks of
    <= 13 etile columns (1664 indices per gather; the final bucket tapers
    to 7) — the fine grain keeps all 4 pairs staggered and fed, and an
    8-deep gather tile pool removes all steady-state sequencer waits.
    num_idxs registers are hoisted (one MOVE per distinct gather size).
  - Per chunk and block of up to 4 etiles (128 edges each): PE transpose of
    the dst rows, matmul with W, then DVE multiply + segmented reduce for
    the per-edge dot product.  Sigmoid is batched on the scalar engine at
    the end; one contiguous DMA writes the [128, total_cols] logits out.
  - Edge j of a core's bucket lives at grid position (j % 128, j // 128);
    the host applies the inverse bucket permutation on the returned grid.

Measured on 8 trn2 cores: 377.6 us (baseline at session start: 1304 us).
"""

import numpy as np

N_NODES = 100000
LATENT = 128
N_EDGES = 625000
N_CORES = 8
SLAB = 25000          # z rows per slab; slab-relative idx fits int16
N_SLABS = 4
BLK = 4               # etile columns per compute block


def _build_nc(n_nodes, slab, bucket_cols, blk=BLK):
    """Trace the SPMD Bass program.

    bucket_cols: list of (slab_src, slab_dst, cols) with cols = per-core
    etile columns for that bucket (cols*128 edges per core).
    """
    import concourse.bacc as bacc
    import concourse.mybir as mybir
    import concourse.tile as tile

    f32 = mybir.dt.float32
    i16 = mybir.dt.int16

    total_cols = sum(c for _, _, c in bucket_cols)
    icols_total = 8 * total_cols  # int16 idx cols (16-wrap) per side

    nc = bacc.Bacc(
        "TRN2", target_bir_lowering=False, debug=False, num_swdge_queues=4
    )

    z = nc.dram_tensor("z", [n_nodes, LATENT], f32, kind="ExternalInput")
    w = nc.dram_tensor("w", [LATENT, LATENT], f32, kind="ExternalInput")
    ident = nc.dram_tensor("ident", [128, 128], f32, kind="ExternalInput")
    src16 = nc.dram_tensor("src16", [128, icols_total], i16, kind="ExternalInput")
    dst16 = nc.dram_tensor("dst16", [128, icols_total], i16, kind="ExternalInput")
    out = nc.dram_tensor("out", [128, total_cols], f32, kind="ExternalOutput")

    with tile.TileContext(nc) as tc:
        with (
            tc.tile_pool(name="const", bufs=1) as constp,
            tc.tile_pool(name="gather", bufs=8) as gatherp,
            tc.tile_pool(name="work", bufs=3) as workp,
            tc.tile_pool(name="psum", bufs=2, space="PSUM") as psump,
            tc.tile_pool(name="outp", bufs=1) as outp,
        ):
            # Index tables first (the first gathers wait on them); each in
            # two halves so early gathers unblock before the tail loads.
            srci = constp.tile([128, icols_total], i16)
            dsti = constp.tile([128, icols_total], i16)
            i0 = min(416, icols_total)   # ~4 chunks of indices
            ih = max(i0, (icols_total // 2) // 8 * 8)
            nc.sync.dma_start(srci[:, :i0], src16[:, :i0])
            nc.scalar.dma_start(dsti[:, :i0], dst16[:, :i0])
            nc.sync.dma_start(srci[:, i0:ih], src16[:, i0:ih])
            nc.scalar.dma_start(dsti[:, i0:ih], dst16[:, i0:ih])
            nc.sync.dma_start(srci[:, ih:], src16[:, ih:])
            nc.scalar.dma_start(dsti[:, ih:], dst16[:, ih:])
            w_sb = constp.tile([128, 128], f32)
            nc.sync.dma_start(w_sb[:], w[:])
            id_sb = constp.tile([128, 128], f32)
            nc.sync.dma_start(id_sb[:], ident[:])

            logits = outp.tile([128, total_cols], f32)

            # Split each bucket into chunks of at most CHUNK_COLS etile
            # columns; every gather goes to the next SWDGE queue round-robin.
            # Each queue q runs its descriptor generation on Q7 core pair
            # (2q, 2q+1), so 4 gathers can generate descriptors
            # concurrently, and the small grain keeps all pairs fed.
            CHUNK_COLS = 13
            nonempty = [bc for bc in bucket_cols if bc[2] > 0]
            chunks = []   # (slab_src, slab_dst, col0, cols)
            col0 = 0
            for bi, (a, d, cols) in enumerate(nonempty):
                # Taper the final bucket into small chunks so the trailing
                # compute drain after the last gathers is short.
                step = 7 if bi == len(nonempty) - 1 else CHUNK_COLS
                for off in range(0, cols, step):
                    cw = min(step, cols - off)
                    chunks.append((a, d, col0 + off, cw))
                col0 += cols

            nidx_regs = {
                k: nc.gpsimd.to_reg(k)
                for k in sorted({cw * 128 for _, _, _, cw in chunks})
            }

            qn = 0
            for a, d, ccol0, cols in chunks:
                n_idx = cols * 128
                ic0, ic1 = 8 * ccol0, 8 * (ccol0 + cols)

                # idx j -> tile[j%128, (j//128)*128 : ...+128]
                zi = gatherp.tile([128, cols * 128], f32, tag="zi")
                nc.gpsimd.dma_gather(
                    out_ap=zi[:].rearrange("p (c f) -> p c f", f=128),
                    in_ap=z[a * slab:(a + 1) * slab, :],
                    idxs_ap=srci[:, ic0:ic1],
                    num_idxs=n_idx,
                    num_idxs_reg=nidx_regs[n_idx],
                    elem_size=128,
                    single_packet=False,
                    queue_num=qn % 4,
                )
                qn += 1
                zj = gatherp.tile([128, cols * 128], f32, tag="zj")
                nc.gpsimd.dma_gather(
                    out_ap=zj[:].rearrange("p (c f) -> p c f", f=128),
                    in_ap=z[d * slab:(d + 1) * slab, :],
                    idxs_ap=dsti[:, ic0:ic1],
                    num_idxs=n_idx,
                    num_idxs_reg=nidx_regs[n_idx],
                    elem_size=128,
                    single_packet=False,
                    queue_num=qn % 4,
                )
                qn += 1

                for b0 in range(0, cols, blk):
                    bw = min(blk, cols - b0)
                    # Transpose bw etiles of zj into one PSUM bank.
                    zjT_ps = psump.tile([128, blk * 128], f32, tag="zjT")
                    for c in range(bw):
                        nc.tensor.transpose(
                            zjT_ps[:, c * 128:(c + 1) * 128],
                            zj[:, (b0 + c) * 128:(b0 + c + 1) * 128],
                            id_sb[:],
                        )
                    zjT_sb = workp.tile([128, blk * 128], f32, tag="zjT_sb")
                    nc.scalar.copy(
                        zjT_sb[:, :bw * 128], zjT_ps[:, :bw * 128]
                    )

                    # u = zj @ W per etile: [e, f] in PSUM.
                    u_ps = psump.tile([128, blk * 128], f32, tag="u")
                    for c in range(bw):
                        nc.tensor.matmul(
                            u_ps[:, c * 128:(c + 1) * 128],
                            lhsT=zjT_sb[:, c * 128:(c + 1) * 128],
                            rhs=w_sb[:],
                            start=True,
                            stop=True,
                        )

                    # Per-edge dot: logits[:, t] = sum_f(u * zi).
                    vscr = workp.tile([128, blk * 128], f32, tag="v")
                    nc.vector.tensor_tensor(
                        out=vscr[:, :bw * 128],
                        in0=u_ps[:, :bw * 128],
                        in1=zi[:, b0 * 128:(b0 + bw) * 128],
                        op=mybir.AluOpType.mult,
                    )
                    nc.vector.tensor_reduce(
                        out=logits[:, ccol0 + b0:ccol0 + b0 + bw],
                        in_=vscr[:, :bw * 128].rearrange(
                            "p (c f) -> p c f", f=128
                        ),
                        axis=mybir.AxisListType.X,
                        op=mybir.AluOpType.add,
                    )

            sig = outp.tile([128, total_cols], f32)
            hh = total_cols // 2
            nc.scalar.activation(
                sig[:, :hh], logits[:, :hh],
                mybir.ActivationFunctionType.Sigmoid,
            )
            nc.sync.dma_start(out[:, :hh], sig[:, :hh])
            nc.scalar.activation(
                sig[:, hh:], logits[:, hh:],
                mybir.ActivationFunctionType.Sigmoid,
            )
            nc.sync.dma_start(out[:, hh:], sig[:, hh:])

    nc.compile()
    return nc


def _wrap16(idx_1d):
    """[n] int16 -> [128, n//16] int16: j at [j%16, j//16], replicated x8."""
    n = idx_1d.shape[0]
    assert n % 16 == 0
    w = idx_1d.reshape(n // 16, 16).T  # [16, n//16]
    return np.ascontiguousarray(np.tile(w, (8, 1)))


def _host_prep(z, edge_index, W, n_nodes=N_NODES, slab=SLAB, n_cores=N_CORES):
    """Bucket + shard the edges. Returns (bucket_cols, in_maps, gather_info)
    where gather_info lets the caller scatter per-core outputs back."""
    z = np.ascontiguousarray(np.asarray(z, dtype=np.float32))
    W = np.ascontiguousarray(np.asarray(W, dtype=np.float32))
    ei = np.asarray(edge_index)
    src = np.asarray(ei[0], dtype=np.int64)
    dst = np.asarray(ei[1], dtype=np.int64)
    n_edges = src.shape[0]
    ident = np.eye(128, dtype=np.float32)

    bucket = (src // slab) * N_SLABS + (dst // slab)
    perm = np.argsort(bucket, kind="stable")
    counts = np.bincount(bucket, minlength=N_SLABS * N_SLABS)

    grain = n_cores * 128
    bucket_cols = []          # (slab_src, slab_dst, per-core cols)
    src_parts, dst_parts = [], []   # per-bucket padded slab-relative indices
    edge_ids = []             # per-bucket padded original edge ids (-1 = pad)
    off = 0
    for b in range(N_SLABS * N_SLABS):
        a, d = divmod(b, N_SLABS)
        n_b = int(counts[b])
        g_b = ((n_b + grain - 1) // grain) * grain
        if n_b == 0:
            bucket_cols.append((a, d, 0))
            continue
        ids = perm[off:off + n_b]
        off += n_b
        s_rel = np.zeros(g_b, dtype=np.int16)
        d_rel = np.zeros(g_b, dtype=np.int16)
        e_id = np.full(g_b, -1, dtype=np.int64)
        s_rel[:n_b] = (src[ids] - a * slab).astype(np.int16)
        d_rel[:n_b] = (dst[ids] - d * slab).astype(np.int16)
        e_id[:n_b] = ids
        src_parts.append(s_rel)
        dst_parts.append(d_rel)
        edge_ids.append(e_id)
        bucket_cols.append((a, d, g_b // grain))

    in_maps = []
    core_edge_ids = []  # per core: concat of bucket slices' edge ids
    for k in range(n_cores):
        s_list, d_list, id_list = [], [], []
        pi = 0
        for (a, d, cols) in bucket_cols:
            if cols == 0:
                continue
            per_core = cols * 128
            sl = slice(k * per_core, (k + 1) * per_core)
            s_list.append(_wrap16(src_parts[pi][sl]))
            d_list.append(_wrap16(dst_parts[pi][sl]))
            id_list.append(edge_ids[pi][sl])
            pi += 1
        in_maps.append({
            "z": z,
            "w": W,
            "ident": ident,
            "src16": np.concatenate(s_list, axis=1),
            "dst16": np.concatenate(d_list, axis=1),
        })
        core_edge_ids.append(np.concatenate(id_list))

    return bucket_cols, in_maps, core_edge_ids


def _unshard(results, core_edge_ids, n_edges):
    """Scatter per-core [128, total_cols] grids back to the full edge order."""
    full = np.zeros(n_edges, dtype=np.float32)
    for k, res in enumerate(results):
        grid = np.asarray(res["out"])            # [128, total_cols]
        flat = grid.T.reshape(-1)                # edge j = t*128 + p
        ids = core_edge_ids[k]
        valid = ids >= 0
        full[ids[valid]] = flat[valid]
    return full


def kernel(z, edge_index, W, _trace=False):
    from concourse.bass_utils import run_bass_kernel_spmd

    bucket_cols, in_maps, core_edge_ids = _host_prep(z, edge_index, W)
    nc = _build_nc(N_NODES, SLAB, bucket_cols)
    res = run_bass_kernel_spmd(
        nc, in_maps, core_ids=list(range(N_CORES)), trace=_trace
    )
    n_edges = np.asarray(edge_index).shape[1]
    full = _unshard(res.results, core_edge_ids, n_edges)
    if _trace:
        kernel.last_results = res
    return full

